# revision 22
# baseline (speedup 1.0000x reference)
"""DeepAR (2-layer LSTM, HID=128) forward on 8 Trainium2 NeuronCores.

Sharding: pure data parallelism. Batch 2048 -> 256 rows per core; LSTM
weights / embedding products replicated; no cross-device communication.

Device kernel layout ("gates on partitions", two 128-wide batch chains):
  - The per-core batch of 256 runs as TWO independent 128-row recurrence
    chains staggered by the Tile scheduler to hide per-step dependency
    latency; the scalar (ACT) engine is the bottleneck (~90% busy).
  - State tiles are [dim=128 partitions, batch=128 free]. Gate
    pre-activations accumulate in PSUM as [128, 4*128] (chunks i,f,g,o
    along the free dim):
        psum[:, 128m:128m+128] = Wx_m^T.T @ x_t  +  Wh_m^T.T @ h_{t-1}
    and one sigmoid activation covers all four gates.
  - Both layers' biases are deposited by K=4 "indicator" matmuls that
    also open the PSUM accumulation group, so all x-matmuls are
    start=False and freely reorderable around the h-dependent ones
    (the PE runs them while the recurrence waits on h).
  - g-gate rows of every weight/bias are pre-scaled by 2 on the host so
    the sigmoid covers the g gate too (tanh(g) = 2*sig(2g)-1, fixed up on
    the vector engine's 4x-mode tensor_scalar).
  - tanh(c2(t-1)) + the h2(t-1) write are software-pipelined across the
    loop boundary: the tanh fires right after sig1(t) as ACT filler while
    the layer-1 DVE cell chain produces c1(t), so the in-order ACT stream
    per chain is [sig1, tanh2(prev), tanh1, sig2] with no serializing
    tail; the h2 write is emitted after the c1 chain so it cannot delay
    c1 in the in-order DVE queue.
  - The first input chunk is small (4 steps) and its DMA is issued ahead
    of the weight loads so the recurrence starts ~8us earlier.
  - h2 history is kept in SBUF; every other step a [14,512] projection
    runs on PE, is staged out of PSUM on DVE (GPSIMD cannot access PSUM),
    and DMA-packed; the softplus + denorm epilogue runs at the end.

kernel(**inputs) is self-contained: hardcodes shapes, shards, compiles via
bass/Tile, runs on cores 0-7 through bass_utils.run_bass_kernel_spmd, and
reassembles the full [2048, 192, 14] float32 output.
"""

import math

import numpy as np
import ml_dtypes

import concourse.bass as bass
import concourse.mybir as mybir
from concourse.tile import TileContext
from concourse import bacc, bass_utils

F32 = mybir.dt.float32
BF16 = mybir.dt.bfloat16
AF = mybir.ActivationFunctionType
ALU = mybir.AluOpType

# Model dims (hardcoded from the problem spec)
B = 2048
SEQ = 168
PRED = 24
W = SEQ + PRED          # 192
TGT = 7
TNUM = 4
TCAT = 3
MNUM = 2
MCAT = 2
COV = 4
HID = 128
INP = 65                # 7 + 28 + 4 + 26
INPX = INP              # no ones row: L1 bias rides an indicator matmul
NCORES = 8
BS = B // NCORES        # 256 batch rows per core
CB = 128                # per-chain batch width
NG = 4 * CB             # 512: four gate chunks along psum free dim
NGR = NG + CB           # 640: gates + tanh-rider tail
XCH = 24                # timesteps per input-chunk DMA
NPROJ = W * BS // 512   # 96 projection chunks of [14, 512]
PGRP = 16               # chunks per partition-group in the packed output
PCC = NPROJ // PGRP     # 6 free-dim column groups

_CACHE = {}


# --------------------------------------------------------------------------
# host-side preprocessing
# --------------------------------------------------------------------------

def _host_prep(inputs):
    ge = inputs["given_enc"].astype(np.float32)
    xe = inputs["x_enc"].astype(np.float32)
    xm = inputs["x_mark_enc"].astype(np.float32)
    mx = inputs["meta_x"].astype(np.float32)
    tembs = [inputs["temb0"], inputs["temb1"], inputs["temb2"]]
    membs = [inputs["memb0"], inputs["memb1"]]

    # categorical embedding gathers
    ge_cat = [tembs[i][ge[:, :, TNUM + i].astype(np.int32)] for i in range(TCAT)]
    mx_cat = [membs[i][mx[:, MNUM + i].astype(np.int32)] for i in range(MCAT)]

    # instance norm over the time axis
    norm_mean = xe.mean(axis=1, keepdims=True)                 # [B,1,7]
    xc = xe - norm_mean
    norm_std = np.sqrt((xc * xc).mean(axis=1, keepdims=True) + 1e-5)
    xn = xc / norm_std

    # teacher forcing shift
    idx = np.clip(np.arange(W) - 1, 0, SEQ - 1)
    prev_y = xn[:, idx, :]                                     # [B,W,7]

    mx_embed = np.concatenate([mx[:, :MNUM]] + mx_cat, axis=-1)   # [B,26]
    mx_b = np.broadcast_to(mx_embed[:, None, :], (B, W, mx_embed.shape[-1]))
    inp = np.concatenate(
        [prev_y, ge[:, :, :TNUM]] + ge_cat + [xm, mx_b], axis=-1
    )                                                          # [B,W,65]
    return inp, norm_mean[:, 0, :], norm_std[:, 0, :]          # means/stds [B,7]


def _gscale(wT):
    """Scale the g-gate block (rows 2*HID:3*HID of the gate dim) by 2.
    wT is [K, 4*HID] (gate dim along columns)."""
    w = wT.copy()
    w[:, 2 * HID:3 * HID] *= 2.0
    return w


def _host_weights(inputs):
    bf = ml_dtypes.bfloat16
    w = {}
    w["wih0"] = _gscale(inputs["Wih0"].T).astype(bf)           # [65, 512]
    w["whh0"] = _gscale(inputs["Whh0"].T).astype(bf)           # [128, 512]
    w["wih1"] = _gscale(inputs["Wih1"].T).astype(bf)           # [128, 512]
    w["whh1"] = _gscale(inputs["Whh1"].T).astype(bf)           # [128, 512]
    b1 = _gscale((inputs["bih0"] + inputs["bhh0"])[None, :])[0]  # [512]
    b2 = _gscale((inputs["bih1"] + inputs["bhh1"])[None, :])[0]  # [512]
    w["b1all"] = b1.reshape(4, HID).astype(bf)                 # [4,128]
    w["b2all"] = b2.reshape(4, HID).astype(bf)                 # [4,128]
    ind = np.zeros((4, NG), np.float32)
    for k in range(4):
        ind[k, CB * k:CB * (k + 1)] = 1.0
    w["ind"] = ind.astype(bf)
    w["wms"] = np.concatenate([inputs["Wm"], inputs["Ws"]], axis=0).T.astype(bf)  # [128,14]
    return w


# --------------------------------------------------------------------------
# device kernel builder
# --------------------------------------------------------------------------

def build_module(nsteps=W):
    # Bacc (not raw Bass): its compile() runs move_matmul_waits_to_ldweights
    # and generate_event_semaphores, which walrus needs (max 1 wait/inst).
    nc = bacc.Bacc("TRN2", target_bir_lowering=False, debug=False,
                   enable_asserts=False, num_devices=NCORES)
    nproj = nsteps * BS // 512
    pcc = max(1, nproj // PGRP)
    ncols = nsteps * BS

    inp_d = nc.dram_tensor("inp", [INPX, ncols], BF16, kind="ExternalInput").ap()
    wih0_d = nc.dram_tensor("wih0", [INPX, 4 * HID], BF16, kind="ExternalInput").ap()
    whh0_d = nc.dram_tensor("whh0", [HID, 4 * HID], BF16, kind="ExternalInput").ap()
    wih1_d = nc.dram_tensor("wih1", [HID, 4 * HID], BF16, kind="ExternalInput").ap()
    whh1_d = nc.dram_tensor("whh1", [HID, 4 * HID], BF16, kind="ExternalInput").ap()
    b1all_d = nc.dram_tensor("b1all", [4, HID], BF16, kind="ExternalInput").ap()
    b2all_d = nc.dram_tensor("b2all", [4, HID], BF16, kind="ExternalInput").ap()
    ind_d = nc.dram_tensor("ind", [4, NG], BF16, kind="ExternalInput").ap()
    wms_d = nc.dram_tensor("wms", [HID, 2 * TGT], BF16, kind="ExternalInput").ap()
    stdp_d = nc.dram_tensor("stdp", [TGT * PGRP, BS], F32, kind="ExternalInput").ap()
    meanp_d = nc.dram_tensor("meanp", [TGT * PGRP, BS], F32, kind="ExternalInput").ap()
    bsp_d = nc.dram_tensor("bsp", [TGT * PGRP, 1], F32, kind="ExternalInput").ap()

    means_d = nc.dram_tensor("means", [TGT * PGRP, 512 * pcc], F32,
                             kind="ExternalOutput").ap()
    sigmas_d = nc.dram_tensor("sigmas", [TGT * PGRP, 512 * pcc], F32,
                              kind="ExternalOutput").ap()

    with TileContext(nc) as tc:
        with tc.tile_pool(name="singles", bufs=1) as singles, \
             tc.tile_pool(name="xin", bufs=3) as xpool, \
             tc.tile_pool(name="vec", bufs=2) as vp, \
             tc.tile_pool(name="sig", bufs=2) as sigp, \
             tc.tile_pool(name="h1p", bufs=2) as h1p:

            def load(name, dram, shape, dtype=BF16):
                t = singles.tile(shape, dtype, tag=name)
                nc.sync.dma_start(out=t[:], in_=dram)
                return t

            XC0 = 4    # small first input chunk: the recurrence starts
            x_first = xpool.tile([INPX, XCH * BS], BF16, tag="x")
            nc.sync.dma_start(out=x_first[:, :XC0 * BS],
                              in_=inp_d[:, 0:XC0 * BS])
            ind = load("ind", ind_d, [4, NG])
            b1all = load("b1all", b1all_d, [4, HID])
            wih0 = load("wih0", wih0_d, [INPX, 4 * HID])
            whh0 = load("whh0", whh0_d, [HID, 4 * HID])
            wih1 = load("wih1", wih1_d, [HID, 4 * HID])
            whh1 = load("whh1", whh1_d, [HID, 4 * HID])
            b2all = load("b2all", b2all_d, [4, HID])
            wms = load("wms", wms_d, [HID, 2 * TGT])
            stdp = load("stdp", stdp_d, [TGT * PGRP, BS], F32)
            meanp = load("meanp", meanp_d, [TGT * PGRP, BS], F32)
            bsp = load("bsp", bsp_d, [TGT * PGRP, 1], F32)

            h2_hist = singles.tile([HID, ncols], BF16, tag="h2_hist")
            means_sb = singles.tile([TGT * PGRP, 512 * pcc], F32, tag="means_sb")
            sigraw_sb = singles.tile([TGT * PGRP, 512 * pcc], F32, tag="sigraw_sb")

            import contextlib
            ctx = contextlib.ExitStack()
            # PSUM budget (8 banks): pg1 1 bank x2 bufs x2 chains, pg2
            # 1 bank x1 x2, proj 1 bank x2 -> 8.
            pools = {}
            for ch in (0, 1):
                pools[ch] = dict(
                    pg1=ctx.enter_context(
                        tc.tile_pool(name=f"pg1{ch}", bufs=2, space="PSUM")),
                    pg2=ctx.enter_context(
                        tc.tile_pool(name=f"pg2{ch}", bufs=1, space="PSUM")),
                )
            projp = ctx.enter_context(
                tc.tile_pool(name="proj", bufs=2, space="PSUM"))
            stagep = ctx.enter_context(tc.tile_pool(name="stage", bufs=3))

            state = [dict(h1=None, c1=None, c2=None, so2=None, cpair=None)
                     for _ in range(2)]
            x_tile = None

            def cell_dve(t, s, cprev, cdst, prefix):
                """DVE cell update from a sigmoid tile s (gates at [0:NG]).
                Writes c into the AP cdst; returns the sigmoid_o slice."""
                si, sf = s[:, 0:CB], s[:, CB:2 * CB]
                sg, so = s[:, 2 * CB:3 * CB], s[:, 3 * CB:4 * CB]
                gt = vp.tile([HID, CB], BF16, tag=f"gt{prefix}")
                nc.vector.tensor_scalar(gt[:], sg, 2.0, 1.0,
                                        ALU.mult, ALU.subtract)
                if t == 0:
                    nc.vector.tensor_mul(cdst, si, gt[:])
                else:
                    v = vp.tile([HID, CB], BF16, tag=f"v{prefix}")
                    nc.vector.tensor_mul(v[:], sf, cprev)
                    u = vp.tile([HID, CB], BF16, tag=f"u{prefix}")
                    nc.vector.tensor_mul(u[:], si, gt[:])
                    nc.vector.tensor_add(cdst, u[:], v[:])
                return so

            def proj_pack(c):
                """[14,512] projection of h2 steps (2c, 2c+1) + DMA pack."""
                pp = projp.tile([2 * TGT, 512], F32, tag="pp")
                nc.tensor.matmul(pp[:], wms[:],
                                 h2_hist[:, 512 * c:512 * (c + 1)],
                                 start=True, stop=True)
                g = c % PGRP
                cc = c // PGRP
                dst = slice(512 * cc, 512 * (cc + 1))
                # DMA cannot read PSUM (and GPSIMD cannot either), and
                # compute engines cannot write at unaligned partition bases
                # -> stage at partition 0 on DVE, then DMA into the packed
                # layout.
                stt = stagep.tile([2 * TGT, 512], F32, tag="st")
                nc.vector.tensor_copy(stt[:], pp[:])
                nc.sync.dma_start(
                    out=means_sb[TGT * g:TGT * (g + 1), dst],
                    in_=stt[0:TGT, :])
                nc.sync.dma_start(
                    out=sigraw_sb[TGT * g:TGT * (g + 1), dst],
                    in_=stt[TGT:2 * TGT, :])

            for t in range(nsteps):
                tt_ = t - XC0
                if t == 0:
                    x_tile = x_first
                elif t >= XC0 and tt_ % XCH == 0:
                    nx = min(XCH, nsteps - t)
                    x_tile = xpool.tile([INPX, XCH * BS], BF16, tag="x")
                    nc.sync.dma_start(
                        out=x_tile[:, :nx * BS],
                        in_=inp_d[:, t * BS:(t + nx) * BS])
                xo = (t % XC0 if t < XC0 else tt_ % XCH) * BS

                for ch in (0, 1):
                    st = state[ch]
                    xt = x_tile[:, xo + CB * ch:xo + CB * (ch + 1)]

                    # ---- layer 1 matmuls: indicator bias opens the psum
                    # group, x-parts are start=False and prefireable, the
                    # h-dependent parts come last
                    g1 = pools[ch]["pg1"].tile([HID, NG], F32, tag=f"pg1{ch}")
                    nc.tensor.matmul(g1[:], b1all[:], ind[:],
                                     start=True, stop=False)
                    for m in range(4):
                        sl = slice(CB * m, CB * (m + 1))
                        nc.tensor.matmul(g1[:, sl],
                                         wih0[:, HID * m:HID * (m + 1)], xt,
                                         start=False, stop=(t == 0 and m == 3))
                    if t > 0:
                        for m in range(4):
                            sl = slice(CB * m, CB * (m + 1))
                            nc.tensor.matmul(g1[:, sl],
                                             whh0[:, HID * m:HID * (m + 1)],
                                             st["h1"][:],
                                             start=False, stop=(m == 3))
                    s1 = sigp.tile([HID, NG], BF16, tag=f"s1{ch}")
                    nc.scalar.activation(s1[:], g1[:], AF.Sigmoid)

                    # deferred L2 tail of step t-1: tanh2 fires as ACT
                    # filler while the layer-1 DVE cell chain produces c1(t)
                    if st["so2"] is not None:
                        th2 = vp.tile([HID, CB], BF16, tag=f"th2{ch}")
                        nc.scalar.activation(th2[:], st["c2"], AF.Tanh)

                    # ---- layer-1 cell (ahead of the h2 write in the
                    # in-order DVE queue so c1 is not delayed)
                    pr = st["cpair"]
                    if pr is None:
                        pr = vp.tile([HID, 2 * CB], BF16, tag=f"pr{ch}")
                    so1 = cell_dve(t, s1, st["c1"], pr[:, 0:CB], f"1{ch}")
                    st["c1"] = pr[:, 0:CB]
                    if st["so2"] is not None:
                        nc.vector.tensor_mul(
                            h2_hist[:, (t - 1) * BS + CB * ch:
                                    (t - 1) * BS + CB * (ch + 1)],
                            st["so2"], th2[:])
                        st["so2"] = None
                    tt1 = vp.tile([HID, CB], BF16, tag=f"t1{ch}")
                    nc.scalar.activation(tt1[:], pr[:, 0:CB], AF.Tanh)
                    h1 = h1p.tile([HID, CB], BF16, tag=f"h1{ch}")
                    nc.vector.tensor_mul(h1[:], so1, tt1[:])
                    st["h1"] = h1

                    # ---- layer 2 matmuls: indicator bias + h-part first,
                    # x-part (h1-dependent) last
                    g2 = pools[ch]["pg2"].tile([HID, NG], F32, tag=f"pg2{ch}")
                    nc.tensor.matmul(g2[:], b2all[:], ind[:],
                                     start=True, stop=False)
                    poff = (t - 1) * BS + CB * ch
                    if t > 0:
                        for m in range(4):
                            sl = slice(CB * m, CB * (m + 1))
                            nc.tensor.matmul(
                                g2[:, sl], whh1[:, HID * m:HID * (m + 1)],
                                h2_hist[:, poff:poff + CB],
                                start=False, stop=False)
                    for m in range(4):
                        sl = slice(CB * m, CB * (m + 1))
                        nc.tensor.matmul(g2[:, sl],
                                         wih1[:, HID * m:HID * (m + 1)], h1[:],
                                         start=False, stop=(m == 3))
                    s2 = sigp.tile([HID, NG], BF16, tag=f"s2{ch}")
                    nc.scalar.activation(s2[:], g2[:], AF.Sigmoid)
                    c2t = vp.tile([HID, CB], BF16, tag=f"c2{ch}")
                    so2 = cell_dve(t, s2, st["c2"], c2t[:], f"2{ch}")
                    st["c2"] = c2t[:]
                    st["so2"] = so2

                # ---- projection for the h2 pair completed by the flushes
                if t % 2 == 0 and t >= 2:
                    proj_pack((t - 2) // 2)

            # drain the final deferred tanh2 of each chain
            for ch in (0, 1):
                st = state[ch]
                th2f = vp.tile([HID, CB], BF16, tag=f"thf{ch}")
                nc.scalar.activation(th2f[:], st["c2"], AF.Tanh)
                nc.vector.tensor_mul(
                    h2_hist[:, (nsteps - 1) * BS + CB * ch:
                            (nsteps - 1) * BS + CB * (ch + 1)],
                    st["so2"], th2f[:])
            proj_pack(nsteps // 2 - 1)
            ctx.close()

            if True:
                # ---- epilogue: softplus + denorm ----
                nf = 2 * pcc  # broadcast factor along free dim
                std_bc = stdp[:, :].unsqueeze(1).broadcast_to(
                    [TGT * PGRP, nf, BS])
                mean_bc = meanp[:, :].unsqueeze(1).broadcast_to(
                    [TGT * PGRP, nf, BS])
                # softplus(x+bs) = ln(1 + exp(x+bs)); Softplus itself has no
                # ACT table set, but exp and ln share one.
                sigsp = singles.tile([TGT * PGRP, 512 * pcc], F32, tag="sigsp")
                nc.scalar.activation(sigsp[:], sigraw_sb[:], AF.Exp,
                                     bias=bsp[:, :])
                nc.scalar.activation(sigsp[:], sigsp[:], AF.Ln, bias=1.0)
                nc.vector.tensor_mul(sigsp[:], sigsp[:], std_bc)
                nc.vector.tensor_mul(means_sb[:], means_sb[:], std_bc)
                nc.vector.tensor_add(means_sb[:], means_sb[:], mean_bc)
                nc.sync.dma_start(out=means_d, in_=means_sb[:])
                nc.sync.dma_start(out=sigmas_d, in_=sigsp[:])

    nc.finalize()
    return nc


# --------------------------------------------------------------------------
# top-level entry
# --------------------------------------------------------------------------

def _pack_norm(arr):
    """[BS,7] per-core norm stats -> [112, BS] tiled PGRP times."""
    a = arr.T.astype(np.float32)                 # [7, BS]
    return np.tile(a, (PGRP, 1)).astype(np.float32)


def run(inputs, trace=False, nsteps=W):
    inputs = {k: np.asarray(v) for k, v in inputs.items()}
    inp, nmean, nstd = _host_prep(inputs)
    wts = _host_weights(inputs)
    bf = ml_dtypes.bfloat16

    bm = inputs["bm"].astype(np.float32)
    bs_ = inputs["bs"].astype(np.float32)

    in_maps = []
    for k in range(NCORES):
        bsl = slice(k * BS, (k + 1) * BS)
        # [BS, nsteps, 65] -> [65, nsteps*BS] with col = t*BS + b
        xi = inp[bsl, :nsteps, :]
        xiT = np.ascontiguousarray(xi.transpose(2, 1, 0).reshape(INPX, -1))
        std_c = nstd[bsl]                        # [BS, 7]
        mean_c = nmean[bsl]
        m = dict(wts)
        m["inp"] = xiT.astype(bf)
        m["stdp"] = _pack_norm(std_c)
        # fold bm*std + mean into the additive term
        m["meanp"] = _pack_norm(bm[None, :] * std_c + mean_c)
        m["bsp"] = np.tile(bs_, PGRP)[:, None].astype(np.float32)
        in_maps.append(m)

    key = nsteps
    if key not in _CACHE:
        _CACHE[key] = build_module(nsteps)
    nc = _CACHE[key]

    res = bass_utils.run_bass_kernel_spmd(
        nc, in_maps, core_ids=list(range(NCORES)), trace=False)

    nproj = nsteps * BS // 512
    pcc = max(1, nproj // PGRP)
    out = np.empty((B, nsteps, 2 * TGT), np.float32)
    for k in range(NCORES):
        r = res.results[k]
        for name, off in (("means", 0), ("sigmas", TGT)):
            a = r[name].reshape(PGRP, TGT, pcc, 2, BS)
            # [g, o, cc, tau, b] -> [b, cc, g, tau, o]
            a = a.transpose(4, 2, 0, 3, 1).reshape(BS, nsteps, TGT)
            out[k * BS:(k + 1) * BS, :, off:off + TGT] = a
    return out, res.exec_time_ns


def kernel(**inputs):
    out, _ = run(inputs, trace=False)
    return out


# revision 33
# speedup vs baseline: 1.0013x; 1.0013x over previous
"""DeepAR (2-layer LSTM, HID=128) forward on 8 Trainium2 NeuronCores.

Sharding: pure data parallelism. Batch 2048 -> 256 rows per core; LSTM
weights / embedding products replicated; no cross-device communication.

Device kernel layout ("gates on partitions", two 128-wide batch chains):
  - The per-core batch of 256 runs as TWO independent 128-row recurrence
    chains staggered by the Tile scheduler to hide per-step dependency
    latency; the scalar (ACT) engine is the bottleneck (~90% busy).
  - State tiles are [dim=128 partitions, batch=128 free]. Gate
    pre-activations accumulate in PSUM as [128, 4*128] (chunks i,f,g,o
    along the free dim):
        psum[:, 128m:128m+128] = Wx_m^T.T @ x_t  +  Wh_m^T.T @ h_{t-1}
    and one sigmoid activation covers all four gates.
  - Both layers' biases are deposited by K=4 "indicator" matmuls that
    also open the PSUM accumulation group, so all x-matmuls are
    start=False and freely reorderable around the h-dependent ones
    (the PE runs them while the recurrence waits on h).
  - g-gate rows of every weight/bias are pre-scaled by 2 on the host so
    the sigmoid covers the g gate too (tanh(g) = 2*sig(2g)-1, fixed up on
    the vector engine's 4x-mode tensor_scalar).
  - tanh(c2(t-1)) + the h2(t-1) write are software-pipelined across the
    loop boundary: the tanh fires right after sig1(t) as ACT filler while
    the layer-1 DVE cell chain produces c1(t), so the in-order ACT stream
    per chain is [sig1, tanh2(prev), tanh1, sig2] with no serializing
    tail; the h2 write is emitted after the c1 chain so it cannot delay
    c1 in the in-order DVE queue.
  - The first input chunk is small (4 steps) and its DMA is issued ahead
    of the weight loads so the recurrence starts ~8us earlier.
  - h2 history is kept in SBUF; every other step a [14,512] projection
    runs on PE, is staged out of PSUM on DVE (GPSIMD cannot access PSUM),
    and DMA-packed; the softplus + denorm epilogue runs at the end.

kernel(**inputs) is self-contained: hardcodes shapes, shards, compiles via
bass/Tile, runs on cores 0-7 through bass_utils.run_bass_kernel_spmd, and
reassembles the full [2048, 192, 14] float32 output.
"""

import math

import numpy as np
import ml_dtypes

import concourse.bass as bass
import concourse.mybir as mybir
from concourse.tile import TileContext
from concourse import bacc, bass_utils

F32 = mybir.dt.float32
BF16 = mybir.dt.bfloat16
AF = mybir.ActivationFunctionType
ALU = mybir.AluOpType

# Model dims (hardcoded from the problem spec)
B = 2048
SEQ = 168
PRED = 24
W = SEQ + PRED          # 192
TGT = 7
TNUM = 4
TCAT = 3
MNUM = 2
MCAT = 2
COV = 4
HID = 128
INP = 65                # 7 + 28 + 4 + 26
INPX = INP              # no ones row: L1 bias rides an indicator matmul
NCORES = 8
BS = B // NCORES        # 256 batch rows per core
CB = 128                # per-chain batch width
NG = 4 * CB             # 512: four gate chunks along psum free dim
NGR = NG + CB           # 640: gates + tanh-rider tail
XCH = 16                # timesteps per input-chunk DMA
NPROJ = W * BS // 512   # 96 projection chunks of [14, 512]
PGRP = 16               # chunks per partition-group in the packed output
PCC = NPROJ // PGRP     # 6 free-dim column groups

_CACHE = {}


# --------------------------------------------------------------------------
# host-side preprocessing
# --------------------------------------------------------------------------

def _host_prep(inputs):
    ge = inputs["given_enc"].astype(np.float32)
    xe = inputs["x_enc"].astype(np.float32)
    xm = inputs["x_mark_enc"].astype(np.float32)
    mx = inputs["meta_x"].astype(np.float32)
    tembs = [inputs["temb0"], inputs["temb1"], inputs["temb2"]]
    membs = [inputs["memb0"], inputs["memb1"]]

    # categorical embedding gathers
    ge_cat = [tembs[i][ge[:, :, TNUM + i].astype(np.int32)] for i in range(TCAT)]
    mx_cat = [membs[i][mx[:, MNUM + i].astype(np.int32)] for i in range(MCAT)]

    # instance norm over the time axis
    norm_mean = xe.mean(axis=1, keepdims=True)                 # [B,1,7]
    xc = xe - norm_mean
    norm_std = np.sqrt((xc * xc).mean(axis=1, keepdims=True) + 1e-5)
    xn = xc / norm_std

    # teacher forcing shift
    idx = np.clip(np.arange(W) - 1, 0, SEQ - 1)
    prev_y = xn[:, idx, :]                                     # [B,W,7]

    mx_embed = np.concatenate([mx[:, :MNUM]] + mx_cat, axis=-1)   # [B,26]
    mx_b = np.broadcast_to(mx_embed[:, None, :], (B, W, mx_embed.shape[-1]))
    inp = np.concatenate(
        [prev_y, ge[:, :, :TNUM]] + ge_cat + [xm, mx_b], axis=-1
    )                                                          # [B,W,65]
    return inp, norm_mean[:, 0, :], norm_std[:, 0, :]          # means/stds [B,7]


def _gscale(wT):
    """Scale the g-gate block (rows 2*HID:3*HID of the gate dim) by 2.
    wT is [K, 4*HID] (gate dim along columns)."""
    w = wT.copy()
    w[:, 2 * HID:3 * HID] *= 2.0
    return w


def _host_weights(inputs):
    bf = ml_dtypes.bfloat16
    w = {}
    w["wih0"] = _gscale(inputs["Wih0"].T).astype(bf)           # [65, 512]
    w["whh0"] = _gscale(inputs["Whh0"].T).astype(bf)           # [128, 512]
    w["wih1"] = _gscale(inputs["Wih1"].T).astype(bf)           # [128, 512]
    w["whh1"] = _gscale(inputs["Whh1"].T).astype(bf)           # [128, 512]
    b1 = _gscale((inputs["bih0"] + inputs["bhh0"])[None, :])[0]  # [512]
    b2 = _gscale((inputs["bih1"] + inputs["bhh1"])[None, :])[0]  # [512]
    w["b1all"] = b1.reshape(4, HID).astype(bf)                 # [4,128]
    w["b2all"] = b2.reshape(4, HID).astype(bf)                 # [4,128]
    # (packed into indb below)
    ind = np.zeros((4, NG), np.float32)
    for k in range(4):
        ind[k, CB * k:CB * (k + 1)] = 1.0
    w["indb"] = np.concatenate(
        [ind.astype(bf), w.pop("b1all"), w.pop("b2all")], axis=1)
    w["wms"] = np.concatenate([inputs["Wm"], inputs["Ws"]], axis=0).T.astype(bf)  # [128,14]
    return w


# --------------------------------------------------------------------------
# device kernel builder
# --------------------------------------------------------------------------

def build_module(nsteps=W):
    # Bacc (not raw Bass): its compile() runs move_matmul_waits_to_ldweights
    # and generate_event_semaphores, which walrus needs (max 1 wait/inst).
    nc = bacc.Bacc("TRN2", target_bir_lowering=False, debug=False,
                   enable_asserts=False, num_devices=NCORES)
    nproj = nsteps * BS // 512
    pcc = max(1, nproj // PGRP)
    ncols = nsteps * BS

    inp_d = nc.dram_tensor("inp", [INPX, ncols], BF16, kind="ExternalInput").ap()
    wih0_d = nc.dram_tensor("wih0", [INPX, 4 * HID], BF16, kind="ExternalInput").ap()
    whh0_d = nc.dram_tensor("whh0", [HID, 4 * HID], BF16, kind="ExternalInput").ap()
    wih1_d = nc.dram_tensor("wih1", [HID, 4 * HID], BF16, kind="ExternalInput").ap()
    whh1_d = nc.dram_tensor("whh1", [HID, 4 * HID], BF16, kind="ExternalInput").ap()
    # small constants packed into single DMAs: [ind | b1all | b2all]
    # along the free dim, and [stdp | meanp | bsp] likewise
    indb_d = nc.dram_tensor("indb", [4, NG + 2 * HID], BF16,
                            kind="ExternalInput").ap()
    wms_d = nc.dram_tensor("wms", [HID, 2 * TGT], BF16, kind="ExternalInput").ap()
    normp_d = nc.dram_tensor("normp", [TGT * PGRP, 2 * BS + 1], F32,
                             kind="ExternalInput").ap()

    means_d = nc.dram_tensor("means", [TGT * PGRP, 512 * pcc], F32,
                             kind="ExternalOutput").ap()
    sigmas_d = nc.dram_tensor("sigmas", [TGT * PGRP, 512 * pcc], F32,
                              kind="ExternalOutput").ap()

    with TileContext(nc) as tc:
        with tc.tile_pool(name="singles", bufs=1) as singles, \
             tc.tile_pool(name="xin", bufs=3) as xpool, \
             tc.tile_pool(name="vec", bufs=2) as vp, \
             tc.tile_pool(name="sig", bufs=3) as sigp, \
             tc.tile_pool(name="h1p", bufs=2) as h1p:

            def load(name, dram, shape, dtype=BF16):
                t = singles.tile(shape, dtype, tag=name)
                nc.sync.dma_start(out=t[:], in_=dram)
                return t

            XC0 = 4    # small first input chunk: the recurrence starts
            x_first = xpool.tile([INPX, XCH * BS], BF16, tag="x")
            nc.sync.dma_start(out=x_first[:, :XC0 * BS],
                              in_=inp_d[:, 0:XC0 * BS])
            indb = load("indb", indb_d, [4, NG + 2 * HID])
            ind = indb[:, 0:NG]
            b1all = indb[:, NG:NG + HID]
            b2all = indb[:, NG + HID:NG + 2 * HID]
            wih0 = load("wih0", wih0_d, [INPX, 4 * HID])
            whh0 = load("whh0", whh0_d, [HID, 4 * HID])
            wih1 = load("wih1", wih1_d, [HID, 4 * HID])
            whh1 = load("whh1", whh1_d, [HID, 4 * HID])
            wms = load("wms", wms_d, [HID, 2 * TGT])
            normp = load("normp", normp_d, [TGT * PGRP, 2 * BS + 1], F32)
            stdp = normp[:, 0:BS]
            meanp = normp[:, BS:2 * BS]
            bsp = normp[:, 2 * BS:2 * BS + 1]

            h2_hist = singles.tile([HID, ncols], BF16, tag="h2_hist")
            means_sb = singles.tile([TGT * PGRP, 512 * pcc], F32, tag="means_sb")
            sigraw_sb = singles.tile([TGT * PGRP, 512 * pcc], F32, tag="sigraw_sb")

            import contextlib
            ctx = contextlib.ExitStack()
            # PSUM budget (8 banks): pg1 1 bank x2 bufs x2 chains, pg2
            # 1 bank x1 x2, proj 1 bank x2 -> 8.
            pools = {}
            for ch in (0, 1):
                pools[ch] = dict(
                    pg1=ctx.enter_context(
                        tc.tile_pool(name=f"pg1{ch}", bufs=1, space="PSUM")),
                    pg2=ctx.enter_context(
                        tc.tile_pool(name=f"pg2{ch}", bufs=2, space="PSUM")),
                )
            projp = ctx.enter_context(
                tc.tile_pool(name="proj", bufs=2, space="PSUM"))
            stagep = ctx.enter_context(tc.tile_pool(name="stage", bufs=3))

            state = [dict(h1=None, c1=None, c2=None, so2=None, cpair=None)
                     for _ in range(2)]
            x_tile = None

            def cell_dve(t, s, cprev, cdst, prefix):
                """DVE cell update from a sigmoid tile s (gates at [0:NG]).
                Writes c into the AP cdst; returns the sigmoid_o slice."""
                si, sf = s[:, 0:CB], s[:, CB:2 * CB]
                sg, so = s[:, 2 * CB:3 * CB], s[:, 3 * CB:4 * CB]
                gt = vp.tile([HID, CB], BF16, tag=f"gt{prefix}")
                nc.vector.tensor_scalar(gt[:], sg, 2.0, 1.0,
                                        ALU.mult, ALU.subtract)
                if t == 0:
                    nc.vector.tensor_mul(cdst, si, gt[:])
                else:
                    v = vp.tile([HID, CB], BF16, tag=f"v{prefix}")
                    nc.vector.tensor_mul(v[:], sf, cprev)
                    u = vp.tile([HID, CB], BF16, tag=f"u{prefix}")
                    nc.vector.tensor_mul(u[:], si, gt[:])
                    nc.vector.tensor_add(cdst, u[:], v[:])
                return so

            def proj_pack(c):
                """[14,512] projection of h2 steps (2c, 2c+1) + DMA pack."""
                pp = projp.tile([2 * TGT, 512], F32, tag="pp")
                nc.tensor.matmul(pp[:], wms[:],
                                 h2_hist[:, 512 * c:512 * (c + 1)],
                                 start=True, stop=True)
                g = c % PGRP
                cc = c // PGRP
                dst = slice(512 * cc, 512 * (cc + 1))
                # DMA cannot read PSUM (and GPSIMD cannot either), and
                # compute engines cannot write at unaligned partition bases
                # -> stage at partition 0 on DVE, then DMA into the packed
                # layout.
                stt = stagep.tile([2 * TGT, 512], F32, tag="st")
                nc.vector.tensor_copy(stt[:], pp[:])
                nc.sync.dma_start(
                    out=means_sb[TGT * g:TGT * (g + 1), dst],
                    in_=stt[0:TGT, :])
                nc.sync.dma_start(
                    out=sigraw_sb[TGT * g:TGT * (g + 1), dst],
                    in_=stt[TGT:2 * TGT, :])

            for t in range(nsteps):
                tt_ = t - XC0
                if t == 0:
                    x_tile = x_first
                elif t >= XC0 and tt_ % XCH == 0:
                    nx = min(XCH, nsteps - t)
                    x_tile = xpool.tile([INPX, XCH * BS], BF16, tag="x")
                    nc.sync.dma_start(
                        out=x_tile[:, :nx * BS],
                        in_=inp_d[:, t * BS:(t + nx) * BS])
                xo = (t % XC0 if t < XC0 else tt_ % XCH) * BS

                for ch in (0, 1):
                    st = state[ch]
                    xt = x_tile[:, xo + CB * ch:xo + CB * (ch + 1)]

                    # ---- layer 1 matmuls: indicator bias opens the psum
                    # group, x-parts are start=False and prefireable, the
                    # h-dependent parts come last
                    g1 = pools[ch]["pg1"].tile([HID, NG], F32, tag=f"pg1{ch}")
                    nc.tensor.matmul(g1[:], b1all[:], ind[:],
                                     start=True, stop=False)
                    for m in range(4):
                        sl = slice(CB * m, CB * (m + 1))
                        nc.tensor.matmul(g1[:, sl],
                                         wih0[:, HID * m:HID * (m + 1)], xt,
                                         start=False, stop=(t == 0 and m == 3))
                    if t > 0:
                        for m in range(4):
                            sl = slice(CB * m, CB * (m + 1))
                            nc.tensor.matmul(g1[:, sl],
                                             whh0[:, HID * m:HID * (m + 1)],
                                             st["h1"][:],
                                             start=False, stop=(m == 3))
                    s1 = sigp.tile([HID, NG], BF16, tag=f"s1{ch}")
                    nc.scalar.activation(s1[:], g1[:], AF.Sigmoid)

                    # deferred L2 tail of step t-1: tanh2 fires as ACT
                    # filler while the layer-1 DVE cell chain produces c1(t)
                    if st["so2"] is not None:
                        th2 = vp.tile([HID, CB], BF16, tag=f"th2{ch}")
                        nc.scalar.activation(th2[:], st["c2"], AF.Tanh)

                    # ---- layer-1 cell (ahead of the h2 write in the
                    # in-order DVE queue so c1 is not delayed)
                    pr = st["cpair"]
                    if pr is None:
                        pr = vp.tile([HID, 2 * CB], BF16, tag=f"pr{ch}")
                    so1 = cell_dve(t, s1, st["c1"], pr[:, 0:CB], f"1{ch}")
                    st["c1"] = pr[:, 0:CB]
                    if st["so2"] is not None:
                        nc.vector.tensor_mul(
                            h2_hist[:, (t - 1) * BS + CB * ch:
                                    (t - 1) * BS + CB * (ch + 1)],
                            st["so2"], th2[:])
                        st["so2"] = None
                    tt1 = vp.tile([HID, CB], BF16, tag=f"t1{ch}")
                    nc.scalar.activation(tt1[:], pr[:, 0:CB], AF.Tanh)
                    h1 = h1p.tile([HID, CB], BF16, tag=f"h1{ch}")
                    nc.vector.tensor_mul(h1[:], so1, tt1[:])
                    st["h1"] = h1

                    # ---- layer 2 matmuls: indicator bias + h-part first,
                    # x-part (h1-dependent) last
                    g2 = pools[ch]["pg2"].tile([HID, NG], F32, tag=f"pg2{ch}")
                    nc.tensor.matmul(g2[:], b2all, ind,
                                     start=True, stop=False)
                    poff = (t - 1) * BS + CB * ch
                    if t > 0:
                        for m in range(4):
                            sl = slice(CB * m, CB * (m + 1))
                            nc.tensor.matmul(
                                g2[:, sl], whh1[:, HID * m:HID * (m + 1)],
                                h2_hist[:, poff:poff + CB],
                                start=False, stop=False)
                    for m in range(4):
                        sl = slice(CB * m, CB * (m + 1))
                        nc.tensor.matmul(g2[:, sl],
                                         wih1[:, HID * m:HID * (m + 1)], h1[:],
                                         start=False, stop=(m == 3))
                    s2 = sigp.tile([HID, NG], BF16, tag=f"s2{ch}")
                    nc.scalar.activation(s2[:], g2[:], AF.Sigmoid)
                    c2t = vp.tile([HID, CB], BF16, tag=f"c2{ch}")
                    so2 = cell_dve(t, s2, st["c2"], c2t[:], f"2{ch}")
                    st["c2"] = c2t[:]
                    st["so2"] = so2

                # ---- projection for the h2 pair completed by the flushes
                if t % 2 == 0 and t >= 2:
                    proj_pack((t - 2) // 2)

            # drain the final deferred tanh2 of each chain
            for ch in (0, 1):
                st = state[ch]
                th2f = vp.tile([HID, CB], BF16, tag=f"thf{ch}")
                nc.scalar.activation(th2f[:], st["c2"], AF.Tanh)
                nc.vector.tensor_mul(
                    h2_hist[:, (nsteps - 1) * BS + CB * ch:
                            (nsteps - 1) * BS + CB * (ch + 1)],
                    st["so2"], th2f[:])
            proj_pack(nsteps // 2 - 1)
            ctx.close()

            if True:
                # ---- epilogue: softplus + denorm ----
                nf = 2 * pcc  # broadcast factor along free dim
                std_bc = stdp.unsqueeze(1).broadcast_to(
                    [TGT * PGRP, nf, BS])
                mean_bc = meanp.unsqueeze(1).broadcast_to(
                    [TGT * PGRP, nf, BS])
                # softplus(x+bs) = ln(1 + exp(x+bs)); Softplus itself has no
                # ACT table set, but exp and ln share one.
                sigsp = singles.tile([TGT * PGRP, 512 * pcc], F32, tag="sigsp")
                nc.scalar.activation(sigsp[:], sigraw_sb[:], AF.Exp,
                                     bias=bsp)
                nc.scalar.activation(sigsp[:], sigsp[:], AF.Ln, bias=1.0)
                nc.vector.tensor_mul(sigsp[:], sigsp[:], std_bc)
                nc.vector.tensor_mul(means_sb[:], means_sb[:], std_bc)
                nc.vector.tensor_add(means_sb[:], means_sb[:], mean_bc)
                nc.sync.dma_start(out=means_d, in_=means_sb[:])
                nc.sync.dma_start(out=sigmas_d, in_=sigsp[:])

    nc.finalize()
    return nc


# --------------------------------------------------------------------------
# top-level entry
# --------------------------------------------------------------------------

def _pack_norm(arr):
    """[BS,7] per-core norm stats -> [112, BS] tiled PGRP times."""
    a = arr.T.astype(np.float32)                 # [7, BS]
    return np.tile(a, (PGRP, 1)).astype(np.float32)


def run(inputs, trace=False, nsteps=W):
    inputs = {k: np.asarray(v) for k, v in inputs.items()}
    inp, nmean, nstd = _host_prep(inputs)
    wts = _host_weights(inputs)
    bf = ml_dtypes.bfloat16

    bm = inputs["bm"].astype(np.float32)
    bs_ = inputs["bs"].astype(np.float32)

    in_maps = []
    for k in range(NCORES):
        bsl = slice(k * BS, (k + 1) * BS)
        # [BS, nsteps, 65] -> [65, nsteps*BS] with col = t*BS + b
        xi = inp[bsl, :nsteps, :]
        xiT = np.ascontiguousarray(xi.transpose(2, 1, 0).reshape(INPX, -1))
        std_c = nstd[bsl]                        # [BS, 7]
        mean_c = nmean[bsl]
        m = dict(wts)
        m["inp"] = xiT.astype(bf)
        # fold bm*std + mean into the additive term; pack [std|mean|bs]
        m["normp"] = np.concatenate(
            [_pack_norm(std_c),
             _pack_norm(bm[None, :] * std_c + mean_c),
             np.tile(bs_, PGRP)[:, None].astype(np.float32)], axis=1)
        in_maps.append(m)

    key = nsteps
    if key not in _CACHE:
        _CACHE[key] = build_module(nsteps)
    nc = _CACHE[key]

    res = bass_utils.run_bass_kernel_spmd(
        nc, in_maps, core_ids=list(range(NCORES)), trace=False)

    nproj = nsteps * BS // 512
    pcc = max(1, nproj // PGRP)
    out = np.empty((B, nsteps, 2 * TGT), np.float32)
    for k in range(NCORES):
        r = res.results[k]
        for name, off in (("means", 0), ("sigmas", TGT)):
            a = r[name].reshape(PGRP, TGT, pcc, 2, BS)
            # [g, o, cc, tau, b] -> [b, cc, g, tau, o]
            a = a.transpose(4, 2, 0, 3, 1).reshape(BS, nsteps, TGT)
            out[k * BS:(k + 1) * BS, :, off:off + TGT] = a
    return out, res.exec_time_ns


def kernel(**inputs):
    out, _ = run(inputs, trace=False)
    return out


# revision 41
# speedup vs baseline: 1.0052x; 1.0039x over previous
"""DeepAR (2-layer LSTM, HID=128) forward on 8 Trainium2 NeuronCores.

Sharding: pure data parallelism. Batch 2048 -> 256 rows per core; LSTM
weights / embedding products replicated; no cross-device communication.

Device kernel layout ("gates on partitions", two 128-wide batch chains):
  - The per-core batch of 256 runs as TWO independent 128-row recurrence
    chains staggered by the Tile scheduler to hide per-step dependency
    latency; the scalar (ACT) engine is the bottleneck (~90% busy).
  - State tiles are [dim=128 partitions, batch=128 free]. Gate
    pre-activations accumulate in PSUM as [128, 4*128] (chunks i,f,g,o
    along the free dim):
        psum[:, 128m:128m+128] = Wx_m^T.T @ x_t  +  Wh_m^T.T @ h_{t-1}
    and one sigmoid activation covers all four gates.
  - Both layers' biases are deposited by K=4 "indicator" matmuls that
    also open the PSUM accumulation group, so all x-matmuls are
    start=False and freely reorderable around the h-dependent ones
    (the PE runs them while the recurrence waits on h).
  - g-gate rows of every weight/bias are pre-scaled by 2 on the host so
    the sigmoid covers the g gate too (tanh(g) = 2*sig(2g)-1, fixed up on
    the vector engine's 4x-mode tensor_scalar).
  - tanh(c2(t-1)) + the h2(t-1) write are software-pipelined across the
    loop boundary: the tanh fires right after sig1(t) as ACT filler while
    the layer-1 DVE cell chain produces c1(t), so the in-order ACT stream
    per chain is [sig1, tanh2(prev), tanh1, sig2] with no serializing
    tail; the h2 write is emitted after the c1 chain so it cannot delay
    c1 in the in-order DVE queue.
  - The first input chunk is small (4 steps) and its DMA is issued ahead
    of the weight loads so the recurrence starts ~8us earlier.
  - h2 history is kept in SBUF; every other step a [14,512] projection
    runs on PE, is staged out of PSUM on DVE (GPSIMD cannot access PSUM),
    and DMA-packed; the softplus + denorm epilogue runs at the end.

kernel(**inputs) is self-contained: hardcodes shapes, shards, compiles via
bass/Tile, runs on cores 0-7 through bass_utils.run_bass_kernel_spmd, and
reassembles the full [2048, 192, 14] float32 output.
"""

import math

import numpy as np
import ml_dtypes

import concourse.bass as bass
import concourse.mybir as mybir
from concourse.tile import TileContext
from concourse import bacc, bass_utils

F32 = mybir.dt.float32
BF16 = mybir.dt.bfloat16
AF = mybir.ActivationFunctionType
ALU = mybir.AluOpType

# Model dims (hardcoded from the problem spec)
B = 2048
SEQ = 168
PRED = 24
W = SEQ + PRED          # 192
TGT = 7
TNUM = 4
TCAT = 3
MNUM = 2
MCAT = 2
COV = 4
HID = 128
INP = 65                # 7 + 28 + 4 + 26
INPX = INP              # no ones row: L1 bias rides an indicator matmul
NCORES = 8
BS = B // NCORES        # 256 batch rows per core
CB = 128                # per-chain batch width
NG = 4 * CB             # 512: four gate chunks along psum free dim
NGR = NG + CB           # 640: gates + tanh-rider tail
XCH = 16                # timesteps per input-chunk DMA
NPROJ = W * BS // 512   # 96 projection chunks of [14, 512]
PGRP = 16               # chunks per partition-group in the packed output
PCC = NPROJ // PGRP     # 6 free-dim column groups

_CACHE = {}


# --------------------------------------------------------------------------
# host-side preprocessing
# --------------------------------------------------------------------------

def _host_prep(inputs):
    ge = inputs["given_enc"].astype(np.float32)
    xe = inputs["x_enc"].astype(np.float32)
    xm = inputs["x_mark_enc"].astype(np.float32)
    mx = inputs["meta_x"].astype(np.float32)
    tembs = [inputs["temb0"], inputs["temb1"], inputs["temb2"]]
    membs = [inputs["memb0"], inputs["memb1"]]

    # categorical embedding gathers
    ge_cat = [tembs[i][ge[:, :, TNUM + i].astype(np.int32)] for i in range(TCAT)]
    mx_cat = [membs[i][mx[:, MNUM + i].astype(np.int32)] for i in range(MCAT)]

    # instance norm over the time axis
    norm_mean = xe.mean(axis=1, keepdims=True)                 # [B,1,7]
    xc = xe - norm_mean
    norm_std = np.sqrt((xc * xc).mean(axis=1, keepdims=True) + 1e-5)
    xn = xc / norm_std

    # teacher forcing shift
    idx = np.clip(np.arange(W) - 1, 0, SEQ - 1)
    prev_y = xn[:, idx, :]                                     # [B,W,7]

    mx_embed = np.concatenate([mx[:, :MNUM]] + mx_cat, axis=-1)   # [B,26]
    mx_b = np.broadcast_to(mx_embed[:, None, :], (B, W, mx_embed.shape[-1]))
    inp = np.concatenate(
        [prev_y, ge[:, :, :TNUM]] + ge_cat + [xm, mx_b], axis=-1
    )                                                          # [B,W,65]
    return inp, norm_mean[:, 0, :], norm_std[:, 0, :]          # means/stds [B,7]


def _gscale(wT):
    """Scale the g-gate block (rows 2*HID:3*HID of the gate dim) by 2.
    wT is [K, 4*HID] (gate dim along columns)."""
    w = wT.copy()
    w[:, 2 * HID:3 * HID] *= 2.0
    return w


def _host_weights(inputs):
    bf = ml_dtypes.bfloat16
    w = {}
    w["wih0"] = _gscale(inputs["Wih0"].T).astype(bf)           # [65, 512]
    w["whh0"] = _gscale(inputs["Whh0"].T).astype(bf)           # [128, 512]
    w["wih1"] = _gscale(inputs["Wih1"].T).astype(bf)           # [128, 512]
    w["whh1"] = _gscale(inputs["Whh1"].T).astype(bf)           # [128, 512]
    b1 = _gscale((inputs["bih0"] + inputs["bhh0"])[None, :])[0]  # [512]
    b2 = _gscale((inputs["bih1"] + inputs["bhh1"])[None, :])[0]  # [512]
    w["b1all"] = b1.reshape(4, HID).astype(bf)                 # [4,128]
    w["b2all"] = b2.reshape(4, HID).astype(bf)                 # [4,128]
    # (packed into indb below)
    ind = np.zeros((4, NG), np.float32)
    for k in range(4):
        ind[k, CB * k:CB * (k + 1)] = 1.0
    w["indb"] = np.concatenate(
        [ind.astype(bf), w.pop("b1all"), w.pop("b2all")], axis=1)
    w["wms"] = np.concatenate([inputs["Wm"], inputs["Ws"]], axis=0).T.astype(bf)  # [128,14]
    return w


# --------------------------------------------------------------------------
# device kernel builder
# --------------------------------------------------------------------------

def build_module(nsteps=W):
    # Bacc (not raw Bass): its compile() runs move_matmul_waits_to_ldweights
    # and generate_event_semaphores, which walrus needs (max 1 wait/inst).
    nc = bacc.Bacc("TRN2", target_bir_lowering=False, debug=False,
                   enable_asserts=False, num_devices=NCORES)
    nproj = nsteps * BS // 512
    pcc = max(1, nproj // PGRP)
    ncols = nsteps * BS

    inp_d = nc.dram_tensor("inp", [INPX, ncols], BF16, kind="ExternalInput").ap()
    wih0_d = nc.dram_tensor("wih0", [INPX, 4 * HID], BF16, kind="ExternalInput").ap()
    whh0_d = nc.dram_tensor("whh0", [HID, 4 * HID], BF16, kind="ExternalInput").ap()
    wih1_d = nc.dram_tensor("wih1", [HID, 4 * HID], BF16, kind="ExternalInput").ap()
    whh1_d = nc.dram_tensor("whh1", [HID, 4 * HID], BF16, kind="ExternalInput").ap()
    # small constants packed into single DMAs: [ind | b1all | b2all]
    # along the free dim, and [stdp | meanp | bsp] likewise
    indb_d = nc.dram_tensor("indb", [4, NG + 2 * HID], BF16,
                            kind="ExternalInput").ap()
    wms_d = nc.dram_tensor("wms", [HID, 2 * TGT], BF16, kind="ExternalInput").ap()
    normp_d = nc.dram_tensor("normp", [TGT * PGRP, 2 * BS + 1], F32,
                             kind="ExternalInput").ap()

    means_d = nc.dram_tensor("means", [TGT * PGRP, 512 * pcc], F32,
                             kind="ExternalOutput").ap()
    sigmas_d = nc.dram_tensor("sigmas", [TGT * PGRP, 512 * pcc], F32,
                              kind="ExternalOutput").ap()

    with TileContext(nc) as tc:
        with tc.tile_pool(name="singles", bufs=1) as singles, \
             tc.tile_pool(name="xin", bufs=3) as xpool, \
             tc.tile_pool(name="vec", bufs=2) as vp, \
             tc.tile_pool(name="sig", bufs=3) as sigp, \
             tc.tile_pool(name="h1p", bufs=2) as h1p:

            def load(name, dram, shape, dtype=BF16):
                t = singles.tile(shape, dtype, tag=name)
                nc.sync.dma_start(out=t[:], in_=dram)
                return t

            XC0 = 4    # small first input chunk: the recurrence starts
            x_first = xpool.tile([INPX, XCH * BS], BF16, tag="x")
            nc.sync.dma_start(out=x_first[:, :XC0 * BS],
                              in_=inp_d[:, 0:XC0 * BS])
            indb = load("indb", indb_d, [4, NG + 2 * HID])
            ind = indb[:, 0:NG]
            b1all = indb[:, NG:NG + HID]
            b2all = indb[:, NG + HID:NG + 2 * HID]
            wih0 = load("wih0", wih0_d, [INPX, 4 * HID])
            whh0 = load("whh0", whh0_d, [HID, 4 * HID])
            wih1 = load("wih1", wih1_d, [HID, 4 * HID])
            whh1 = load("whh1", whh1_d, [HID, 4 * HID])
            wms = load("wms", wms_d, [HID, 2 * TGT])
            normp = load("normp", normp_d, [TGT * PGRP, 2 * BS + 1], F32)
            stdp = normp[:, 0:BS]
            meanp = normp[:, BS:2 * BS]
            bsp = normp[:, 2 * BS:2 * BS + 1]

            h2_hist = singles.tile([HID, ncols], BF16, tag="h2_hist")
            means_sb = singles.tile([TGT * PGRP, 512 * pcc], F32, tag="means_sb")
            sigraw_sb = singles.tile([TGT * PGRP, 512 * pcc], F32, tag="sigraw_sb")

            import contextlib
            ctx = contextlib.ExitStack()
            # PSUM budget (8 banks): pg1 1 bank x2 bufs x2 chains, pg2
            # 1 bank x1 x2, proj 1 bank x2 -> 8.
            pools = {}
            for ch in (0, 1):
                pools[ch] = dict(
                    pg1=ctx.enter_context(
                        tc.tile_pool(name=f"pg1{ch}", bufs=1, space="PSUM")),
                    pg2=ctx.enter_context(
                        tc.tile_pool(name=f"pg2{ch}", bufs=2, space="PSUM")),
                )
            projp = ctx.enter_context(
                tc.tile_pool(name="proj", bufs=2, space="PSUM"))
            stagep = ctx.enter_context(tc.tile_pool(name="stage", bufs=3))

            state = [dict(h1=None, c1=None, c2=None, so2=None, cpair=None)
                     for _ in range(2)]
            x_tile = None

            def cell_dve(t, s, cprev, cdst, prefix):
                """DVE cell update from a sigmoid tile s (gates at [0:NG]).
                Writes c into the AP cdst; returns the sigmoid_o slice."""
                si, sf = s[:, 0:CB], s[:, CB:2 * CB]
                sg, so = s[:, 2 * CB:3 * CB], s[:, 3 * CB:4 * CB]
                gt = vp.tile([HID, CB], BF16, tag=f"gt{prefix}")
                nc.vector.tensor_scalar(gt[:], sg, 2.0, 1.0,
                                        ALU.mult, ALU.subtract)
                if t == 0:
                    nc.vector.tensor_mul(cdst, si, gt[:])
                else:
                    v = vp.tile([HID, CB], BF16, tag=f"v{prefix}")
                    nc.vector.tensor_mul(v[:], sf, cprev)
                    u = vp.tile([HID, CB], BF16, tag=f"u{prefix}")
                    nc.vector.tensor_mul(u[:], si, gt[:])
                    nc.vector.tensor_add(cdst, u[:], v[:])
                return so

            def proj_pack(c):
                """[14,512] projection of h2 steps (2c, 2c+1) + DMA pack."""
                pp = projp.tile([2 * TGT, 512], F32, tag="pp")
                nc.tensor.matmul(pp[:], wms[:],
                                 h2_hist[:, 512 * c:512 * (c + 1)],
                                 start=True, stop=True)
                g = c % PGRP
                cc = c // PGRP
                dst = slice(512 * cc, 512 * (cc + 1))
                # DMA cannot read PSUM (and GPSIMD cannot either), and
                # compute engines cannot write at unaligned partition bases
                # -> stage at partition 0 on DVE, then DMA into the packed
                # layout.
                stt = stagep.tile([2 * TGT, 512], F32, tag="st")
                nc.vector.tensor_copy(stt[:], pp[:])
                nc.sync.dma_start(
                    out=means_sb[TGT * g:TGT * (g + 1), dst],
                    in_=stt[0:TGT, :])
                nc.sync.dma_start(
                    out=sigraw_sb[TGT * g:TGT * (g + 1), dst],
                    in_=stt[TGT:2 * TGT, :])

            for t in range(nsteps):
                tt_ = t - XC0
                if t == 0:
                    x_tile = x_first
                elif t >= XC0 and tt_ % XCH == 0:
                    nx = min(XCH, nsteps - t)
                    x_tile = xpool.tile([INPX, XCH * BS], BF16, tag="x")
                    nc.sync.dma_start(
                        out=x_tile[:, :nx * BS],
                        in_=inp_d[:, t * BS:(t + nx) * BS])
                xo = (t % XC0 if t < XC0 else tt_ % XCH) * BS

                for ch in (0, 1):
                    st = state[ch]
                    xt = x_tile[:, xo + CB * ch:xo + CB * (ch + 1)]

                    # ---- layer 1 matmuls: indicator bias opens the psum
                    # group, x-parts are start=False and prefireable, the
                    # h-dependent parts come last
                    g1 = pools[ch]["pg1"].tile([HID, NG], F32, tag=f"pg1{ch}")
                    nc.tensor.matmul(g1[:], b1all[:], ind[:],
                                     start=True, stop=False)
                    for m in range(4):
                        sl = slice(CB * m, CB * (m + 1))
                        nc.tensor.matmul(g1[:, sl],
                                         wih0[:, HID * m:HID * (m + 1)], xt,
                                         start=False, stop=(t == 0 and m == 3))
                    if t > 0:
                        for m in range(4):
                            sl = slice(CB * m, CB * (m + 1))
                            nc.tensor.matmul(g1[:, sl],
                                             whh0[:, HID * m:HID * (m + 1)],
                                             st["h1"][:],
                                             start=False, stop=(m == 3))
                    s1 = sigp.tile([HID, NG], BF16, tag=f"s1{ch}")
                    nc.scalar.activation(s1[:], g1[:], AF.Sigmoid)

                    # deferred L2 tail of step t-1: tanh2 fires as ACT
                    # filler while the layer-1 DVE cell chain produces c1(t)
                    if st["so2"] is not None:
                        th2 = vp.tile([HID, CB], BF16, tag=f"th2{ch}")
                        nc.scalar.activation(th2[:], st["c2"], AF.Tanh)

                    # ---- layer-1 cell (ahead of the h2 write in the
                    # in-order DVE queue so c1 is not delayed)
                    pr = st["cpair"]
                    if pr is None:
                        pr = vp.tile([HID, 2 * CB], BF16, tag=f"pr{ch}")
                    so1 = cell_dve(t, s1, st["c1"], pr[:, 0:CB], f"1{ch}")
                    st["c1"] = pr[:, 0:CB]
                    if st["so2"] is not None:
                        # h2 write has ~a quarter-cycle of slack: run it on
                        # the otherwise-idle Pool engine to keep the DVE
                        # queue clear between c1 and the h1 multiply
                        nc.gpsimd.tensor_mul(
                            h2_hist[:, (t - 1) * BS + CB * ch:
                                    (t - 1) * BS + CB * (ch + 1)],
                            st["so2"], th2[:])
                        st["so2"] = None
                    tt1 = vp.tile([HID, CB], BF16, tag=f"t1{ch}")
                    nc.scalar.activation(tt1[:], pr[:, 0:CB], AF.Tanh)
                    h1 = h1p.tile([HID, CB], BF16, tag=f"h1{ch}")
                    nc.vector.tensor_mul(h1[:], so1, tt1[:])
                    st["h1"] = h1

                    # ---- layer 2 matmuls: indicator bias + h-part first,
                    # x-part (h1-dependent) last
                    g2 = pools[ch]["pg2"].tile([HID, NG], F32, tag=f"pg2{ch}")
                    nc.tensor.matmul(g2[:], b2all, ind,
                                     start=True, stop=False)
                    poff = (t - 1) * BS + CB * ch
                    if t > 0:
                        for m in range(4):
                            sl = slice(CB * m, CB * (m + 1))
                            nc.tensor.matmul(
                                g2[:, sl], whh1[:, HID * m:HID * (m + 1)],
                                h2_hist[:, poff:poff + CB],
                                start=False, stop=False)
                    for m in range(4):
                        sl = slice(CB * m, CB * (m + 1))
                        nc.tensor.matmul(g2[:, sl],
                                         wih1[:, HID * m:HID * (m + 1)], h1[:],
                                         start=False, stop=(m == 3))
                    s2 = sigp.tile([HID, NG], BF16, tag=f"s2{ch}")
                    nc.scalar.activation(s2[:], g2[:], AF.Sigmoid)
                    c2t = vp.tile([HID, CB], BF16, tag=f"c2{ch}")
                    so2 = cell_dve(t, s2, st["c2"], c2t[:], f"2{ch}")
                    st["c2"] = c2t[:]
                    st["so2"] = so2

                # ---- projection for the h2 pair completed by the flushes
                if t % 2 == 0 and t >= 2:
                    proj_pack((t - 2) // 2)

            # drain the final deferred tanh2 of each chain
            for ch in (0, 1):
                st = state[ch]
                th2f = vp.tile([HID, CB], BF16, tag=f"thf{ch}")
                nc.scalar.activation(th2f[:], st["c2"], AF.Tanh)
                nc.vector.tensor_mul(
                    h2_hist[:, (nsteps - 1) * BS + CB * ch:
                            (nsteps - 1) * BS + CB * (ch + 1)],
                    st["so2"], th2f[:])
            proj_pack(nsteps // 2 - 1)
            ctx.close()

            if True:
                # ---- epilogue: softplus + denorm ----
                nf = 2 * pcc  # broadcast factor along free dim
                std_bc = stdp.unsqueeze(1).broadcast_to(
                    [TGT * PGRP, nf, BS])
                mean_bc = meanp.unsqueeze(1).broadcast_to(
                    [TGT * PGRP, nf, BS])
                # softplus(x+bs) = ln(1 + exp(x+bs)); Softplus itself has no
                # ACT table set, but exp and ln share one.
                sigsp = singles.tile([TGT * PGRP, 512 * pcc], F32, tag="sigsp")
                nc.scalar.activation(sigsp[:], sigraw_sb[:], AF.Exp,
                                     bias=bsp)
                nc.scalar.activation(sigsp[:], sigsp[:], AF.Ln, bias=1.0)
                nc.vector.tensor_mul(sigsp[:], sigsp[:], std_bc)
                nc.vector.tensor_mul(means_sb[:], means_sb[:], std_bc)
                nc.vector.tensor_add(means_sb[:], means_sb[:], mean_bc)
                nc.sync.dma_start(out=means_d, in_=means_sb[:])
                nc.sync.dma_start(out=sigmas_d, in_=sigsp[:])

    nc.finalize()
    return nc


# --------------------------------------------------------------------------
# top-level entry
# --------------------------------------------------------------------------

def _pack_norm(arr):
    """[BS,7] per-core norm stats -> [112, BS] tiled PGRP times."""
    a = arr.T.astype(np.float32)                 # [7, BS]
    return np.tile(a, (PGRP, 1)).astype(np.float32)


def run(inputs, trace=False, nsteps=W):
    inputs = {k: np.asarray(v) for k, v in inputs.items()}
    inp, nmean, nstd = _host_prep(inputs)
    wts = _host_weights(inputs)
    bf = ml_dtypes.bfloat16

    bm = inputs["bm"].astype(np.float32)
    bs_ = inputs["bs"].astype(np.float32)

    in_maps = []
    for k in range(NCORES):
        bsl = slice(k * BS, (k + 1) * BS)
        # [BS, nsteps, 65] -> [65, nsteps*BS] with col = t*BS + b
        xi = inp[bsl, :nsteps, :]
        xiT = np.ascontiguousarray(xi.transpose(2, 1, 0).reshape(INPX, -1))
        std_c = nstd[bsl]                        # [BS, 7]
        mean_c = nmean[bsl]
        m = dict(wts)
        m["inp"] = xiT.astype(bf)
        # fold bm*std + mean into the additive term; pack [std|mean|bs]
        m["normp"] = np.concatenate(
            [_pack_norm(std_c),
             _pack_norm(bm[None, :] * std_c + mean_c),
             np.tile(bs_, PGRP)[:, None].astype(np.float32)], axis=1)
        in_maps.append(m)

    key = nsteps
    if key not in _CACHE:
        _CACHE[key] = build_module(nsteps)
    nc = _CACHE[key]

    res = bass_utils.run_bass_kernel_spmd(
        nc, in_maps, core_ids=list(range(NCORES)), trace=False)

    nproj = nsteps * BS // 512
    pcc = max(1, nproj // PGRP)
    out = np.empty((B, nsteps, 2 * TGT), np.float32)
    for k in range(NCORES):
        r = res.results[k]
        for name, off in (("means", 0), ("sigmas", TGT)):
            a = r[name].reshape(PGRP, TGT, pcc, 2, BS)
            # [g, o, cc, tau, b] -> [b, cc, g, tau, o]
            a = a.transpose(4, 2, 0, 3, 1).reshape(BS, nsteps, TGT)
            out[k * BS:(k + 1) * BS, :, off:off + TGT] = a
    return out, res.exec_time_ns


def kernel(**inputs):
    out, _ = run(inputs, trace=False)
    return out
